# revision 11
# baseline (speedup 1.0000x reference)
"""Trainium2 Bass kernel for a pre-norm transformer encoder block.

Problem shapes (hardcoded): x [4, 2048, 768], 12 heads x 64, d_ff 3072.

Sharding: 8 cores, no collectives. Core c handles batch b = c // 2 and the
token half h = c % 2 (1024 "own" tokens). Each core receives the full 2048
tokens of its batch (own half first) so it can compute K/V locally; Q and
everything downstream (proj, MLP, output) run on its 1024 own tokens only.

Per-core on-chip schedule (all matmuls in float32r = full PE speed, ~1e-4 rel):
  A. LN1 over 2048 tokens, PE-transpose h -> hT [768, 2048]
  B. QKV: K^T,Q^T feature-major; V token-major staged as [keys, head, 64+1]
     with a ones column (ones-trick: PV matmul also yields softmax sums)
  C. attention per (head, q-chunk): S^T = kT.T @ qT (keys on partitions),
     P = exp(S/8) with no max-subtraction (scores are bounded ~8), O'^T//sums
     via ones-augmented PV + K=1 broadcast matmul for the normalization
  D. proj + residual -> xo, LN2, transpose h2 -> h2T
  E. MLP: g^T = gelu(w1.T @ h2T) feature-major, fc2 accumulated into xo
"""

import os
import sys
import types

import numpy as np

# This image's antenv lacks ``axon_hooks``, so the boot shim can't register
# the NTFF-profiling hook and trace=True silently degrades. Provide the
# registry module with a lazily-built ctypes hook against libaxon_pjrt.so
# (mirrors trn_agent_boot.trn_boot._ntff_profile_via_ctypes).
if "antenv.axon_hooks" not in sys.modules:
    _m = types.ModuleType("antenv.axon_hooks")
    _m._hook = None

    def _build_ctypes_hook():
        import contextlib
        import ctypes

        so_path = "/opt/axon/libaxon_pjrt.so"
        if not os.path.exists(so_path):
            return None
        lib = ctypes.CDLL(so_path)
        if not hasattr(lib, "axon_start_nrt_profile"):
            return None
        lib.axon_start_nrt_profile.argtypes = [
            ctypes.POINTER(ctypes.c_int64), ctypes.c_size_t]
        lib.axon_start_nrt_profile.restype = ctypes.c_int64
        lib.axon_stop_nrt_profile.argtypes = [ctypes.c_char_p]
        lib.axon_stop_nrt_profile.restype = ctypes.c_int64

        @contextlib.contextmanager
        def _hook(output_dir, device_ids):
            import jax
            jax.devices()
            if device_ids:
                ids = (ctypes.c_int64 * len(device_ids))(*device_ids)
                rc = lib.axon_start_nrt_profile(ids, len(device_ids))
            else:
                rc = lib.axon_start_nrt_profile(None, 0)
            if rc != 0:
                raise RuntimeError(f"axon_start_nrt_profile rc={rc}")
            try:
                yield
            finally:
                n = lib.axon_stop_nrt_profile(str(output_dir).encode())
                if n < 0:
                    raise RuntimeError(f"axon_stop_nrt_profile rc={n}")
                print(f"profile: {n} file(s) written to {output_dir}")

        return _hook

    def _set(h, _m=_m):
        _m._hook = h

    def _get(_m=_m):
        if _m._hook is None:
            _m._hook = _build_ctypes_hook()
        return _m._hook

    _m.set_axon_ntff_profile_hook = _set
    _m.get_axon_ntff_profile_hook = _get
    sys.modules["antenv.axon_hooks"] = _m

B, N, C = 4, 2048, 768
HEADS, HD = 12, 64
FF = 4 * C
P = 128
NT = N // P            # 16 token tiles (full context)
QT_ = (N // 2) // P    # 8 own token tiles
CT = C // P            # 6 feature tiles
FT = FF // P           # 24 ff tiles
LN_EPS = 1e-5

_CACHE = {}
LAST_RESULT = None


def _build(has_bpo, has_bo):
    import concourse.bass as bass
    import concourse.mybir as mybir
    import concourse.tile as tile
    from concourse import bacc
    from contextlib import ExitStack

    F32 = mybir.dt.float32
    F32R = mybir.dt.float32r
    AF = mybir.ActivationFunctionType
    OP = mybir.AluOpType

    nc = bacc.Bacc(None, target_bir_lowering=False)

    # ---- DRAM tensors ----
    x_in = nc.dram_tensor("x_in", [N, C], F32, kind="ExternalInput")
    wq = nc.dram_tensor("wq", [CT, P, CT, P], F32R, kind="ExternalInput")
    wk = nc.dram_tensor("wk", [CT, P, CT, P], F32R, kind="ExternalInput")
    wv = nc.dram_tensor("wv", [CT, P, C], F32R, kind="ExternalInput")
    wp = nc.dram_tensor("wp", [CT, P, C], F32R, kind="ExternalInput")
    w1 = nc.dram_tensor("w1", [FT, P, CT, P], F32R, kind="ExternalInput")
    w2 = nc.dram_tensor("w2", [FT, P, C], F32R, kind="ExternalInput")
    qb = nc.dram_tensor("qb", [P, CT], F32, kind="ExternalInput")
    b1v = nc.dram_tensor("b1v", [P, FT], F32, kind="ExternalInput")
    bpo = nc.dram_tensor("bpo", [C], F32, kind="ExternalInput")
    bo = nc.dram_tensor("bo", [C], F32, kind="ExternalInput")
    ident = nc.dram_tensor("ident", [P, P], F32R, kind="ExternalInput")
    ones64 = nc.dram_tensor("ones64", [1, HD], F32R, kind="ExternalInput")
    onesc = nc.dram_tensor("onesc", [P, NT * HEADS], F32R, kind="ExternalInput")
    y = nc.dram_tensor("y", [N // 2, C], F32, kind="ExternalOutput")

    def bcast_rows(t):
        # DRAM [C] -> AP broadcasting along 128 partitions
        return bass.AP(tensor=t.tensor, offset=t.offset, ap=[[0, P], list(t.ap[0])])

    def layernorm(pool, xt, ht, t_eps):
        # xt [128, 768] f32 -> ht [128, 768] f32r, normalized (no scale/bias)
        stats = pool.tile([P, 3, nc.vector.BN_STATS_DIM], F32, tag="ln_stats")
        for sg in range(3):
            nc.vector.bn_stats(out=stats[:, sg], in_=xt[:, sg * 256:(sg + 1) * 256])
        mv = pool.tile([P, nc.vector.BN_AGGR_DIM], F32, tag="ln_mv")
        nc.vector.bn_aggr(out=mv[:], in_=stats[:])
        std = pool.tile([P, 1], F32, tag="ln_std")
        nc.scalar.activation(out=std[:], in_=mv[:, 1:2], func=AF.Sqrt, bias=t_eps[:])
        rstd = pool.tile([P, 1], F32, tag="ln_rstd")
        nc.vector.reciprocal(out=rstd[:], in_=std[:])
        with nc.allow_low_precision(reason="fp32r rounding for matmul input"):
            nc.vector.tensor_scalar(
                out=ht[:], in0=xt[:], scalar1=mv[:, 0:1], scalar2=rstd[:],
                op0=OP.subtract, op1=OP.mult)

    with tile.TileContext(nc) as tc, ExitStack() as top:
        consts = top.enter_context(tc.tile_pool(name="consts", bufs=1))
        t_id = consts.tile([P, P], F32R)
        nc.sync.dma_start(t_id[:], ident[:])
        t_ones64 = consts.tile([1, HD], F32R)
        nc.sync.dma_start(t_ones64[:], ones64[:])
        t_qb = consts.tile([P, CT], F32)
        nc.sync.dma_start(t_qb[:], qb[:])
        t_b1 = consts.tile([P, FT], F32)
        nc.sync.dma_start(t_b1[:], b1v[:])
        t_eps = consts.tile([P, 1], F32)
        nc.vector.memset(t_eps[:], LN_EPS)
        t_bpo = t_bo = None
        if has_bpo:
            t_bpo = consts.tile([P, C], F32)
            nc.sync.dma_start(t_bpo[:], bcast_rows(bpo[:]))
        if has_bo:
            t_bo = consts.tile([P, C], F32)
            nc.sync.dma_start(t_bo[:], bcast_rows(bo[:]))

        s_kqv = ExitStack()   # closes after attention
        s_OT = ExitStack()    # closes after proj
        s_xo = ExitStack()    # closes at end
        s_h2T = ExitStack()   # closes after fc1
        s_gT = ExitStack()    # closes at end
        top.enter_context(s_gT)
        top.enter_context(s_xo)

        pool_kqv = s_kqv.enter_context(tc.tile_pool(name="kqv", bufs=1))
        t_KT = pool_kqv.tile([P, CT, N], F32R)        # K^T feature-major
        t_QT = pool_kqv.tile([P, CT, N // 2], F32R)   # Q^T own tokens
        t_V = pool_kqv.tile([P, NT, HEADS, HD + 1], F32R)  # V + ones col

        # ---------- Phase A+B: LN1 + transpose + QKV ----------
        with ExitStack() as ph:
            lnp = ph.enter_context(tc.tile_pool(name="ln1", bufs=2))
            wst = ph.enter_context(tc.tile_pool(name="wst", bufs=2))
            hTp = ph.enter_context(tc.tile_pool(name="hT", bufs=1))
            psA = ph.enter_context(tc.tile_pool(name="psA", bufs=3, space="PSUM"))
            psTr = ph.enter_context(tc.tile_pool(name="psTr", bufs=2, space="PSUM"))

            # ones columns of the V staging buffer, one DMA
            nc.sync.dma_start(
                t_V[:, :, :, HD:HD + 1],
                onesc[:].rearrange("p (t h) -> p t h", t=NT)[:, :, :, None])

            t_wv = wst.tile([P, CT, C], F32R, tag="wv")
            nc.sync.dma_start(t_wv[:], wv[:].rearrange("c p n -> p c n"))

            for g in range(2):  # token groups of 1024 (g=0: own tokens)
                t_hT = hTp.tile([P, CT, N // 2], F32R, tag="hT")
                for tt in range(QT_):
                    t = g * QT_ + tt
                    xt = lnp.tile([P, C], F32, tag="xt")
                    nc.sync.dma_start(xt[:], x_in[t * P:(t + 1) * P, :])
                    ht = lnp.tile([P, C], F32R, tag="ht")
                    layernorm(lnp, xt, ht, t_eps)
                    for c in range(CT):
                        pst = psTr.tile([P, P], F32R, tag="tr")
                        nc.tensor.transpose(pst[:], ht[:, c * P:(c + 1) * P], t_id[:])
                        with nc.allow_low_precision(reason="fp32r"):
                            nc.vector.tensor_copy(
                                out=t_hT[:, c, tt * P:(tt + 1) * P], in_=pst[:])
                # K^T (this token group's columns)
                for f in range(CT):
                    t_wk = wst.tile([P, CT, P], F32R, tag="wk")
                    nc.sync.dma_start(t_wk[:], wk[f])
                    for tc2 in range(2):
                        ps = psA.tile([P, 512], F32, tag="mm")
                        for c in range(CT):
                            nc.tensor.matmul(
                                ps[:], t_wk[:, c], t_hT[:, c, tc2 * 512:(tc2 + 1) * 512],
                                start=(c == 0), stop=(c == CT - 1))
                        with nc.allow_low_precision(reason="fp32r"):
                            nc.vector.tensor_copy(
                                out=t_KT[:, f, g * 1024 + tc2 * 512:g * 1024 + (tc2 + 1) * 512],
                                in_=ps[:])
                # Q^T (own tokens only)
                if g == 0:
                    for f in range(CT):
                        t_wq = wst.tile([P, CT, P], F32R, tag="wq")
                        nc.sync.dma_start(t_wq[:], wq[f])
                        for tc2 in range(2):
                            ps = psA.tile([P, 512], F32, tag="mm")
                            for c in range(CT):
                                nc.tensor.matmul(
                                    ps[:], t_wq[:, c], t_hT[:, c, tc2 * 512:(tc2 + 1) * 512],
                                    start=(c == 0), stop=(c == CT - 1))
                            with nc.allow_low_precision(reason="fp32r"):
                                nc.vector.tensor_scalar(
                                    out=t_QT[:, f, tc2 * 512:(tc2 + 1) * 512], in0=ps[:],
                                    scalar1=t_qb[:, f:f + 1], scalar2=None, op0=OP.add)
                # V token-major, staged per head with ones column
                for tt in range(QT_):
                    t = g * QT_ + tt
                    for nc2 in range(2):
                        ps = psA.tile([P, 384], F32, tag="mmv")
                        for c in range(CT):
                            nc.tensor.matmul(
                                ps[:], t_hT[:, c, tt * P:(tt + 1) * P],
                                t_wv[:, c, nc2 * 384:(nc2 + 1) * 384],
                                start=(c == 0), stop=(c == CT - 1))
                        with nc.allow_low_precision(reason="fp32r"):
                            nc.vector.tensor_copy(
                                out=t_V[:, t, 6 * nc2:6 * nc2 + 6, :HD],
                                in_=ps[:].rearrange("p (h d) -> p h d", d=HD))

        # ---------- Phase C: attention ----------
        pool_OT = s_OT.enter_context(tc.tile_pool(name="OT", bufs=1, side="right"))
        t_OT = pool_OT.tile([P, CT, N // 2], F32R)
        with ExitStack() as ph:
            ptp = ph.enter_context(tc.tile_pool(name="pt", bufs=3))
            rbp = ph.enter_context(tc.tile_pool(name="rb", bufs=2))
            psS = ph.enter_context(tc.tile_pool(name="psS", bufs=2, space="PSUM"))
            psO = ph.enter_context(tc.tile_pool(name="psO", bufs=1, space="PSUM"))
            psB = ph.enter_context(tc.tile_pool(name="psB", bufs=1, space="PSUM"))
            for hp in range(HEADS // 2):
                for qc in range(2):
                    qs = slice(qc * 512, (qc + 1) * 512)
                    pso = {}
                    for sub in range(2):
                        pso[sub] = psO.tile(
                            [HD + 1, 512], F32, tag=f"o{sub}", name=f"pso{sub}")
                    for kt in range(NT):
                        for sub in range(2):
                            h = 2 * hp + sub
                            off = sub * HD
                            ps = psS.tile([P, 512], F32, tag=f"s{sub}")
                            nc.tensor.matmul(
                                ps[:], t_KT[off:off + HD, hp, kt * P:(kt + 1) * P],
                                t_QT[off:off + HD, hp, qs], start=True, stop=True)
                            pt = ptp.tile([P, 512], F32R, tag=f"pt{sub}")
                            nc.scalar.activation(
                                out=pt[:], in_=ps[:], func=AF.Exp, scale=0.125)
                            nc.tensor.matmul(
                                pso[sub][:], t_V[:, kt, h, :], pt[:],
                                start=(kt == 0), stop=(kt == NT - 1))
                    for sub in range(2):
                        off = sub * HD
                        r = rbp.tile([1, 512], F32R, tag="r")
                        with nc.allow_low_precision(reason="fp32r"):
                            nc.vector.reciprocal(out=r[:], in_=pso[sub][HD:HD + 1, :])
                        pb = psB.tile([HD, 512], F32, tag=f"b{sub}")
                        nc.tensor.matmul(pb[:], t_ones64[:], r[:], start=True, stop=True)
                        rb = rbp.tile([HD, 512], F32, tag=f"rb{sub}")
                        nc.scalar.copy(out=rb[:], in_=pb[:])
                        with nc.allow_low_precision(reason="fp32r"):
                            nc.vector.tensor_tensor(
                                out=t_OT[off:off + HD, hp, qs], in0=pso[sub][:HD, :],
                                in1=rb[:], op=OP.mult)

        # ---------- Phase D: proj + residual + LN2 + transpose ----------
        s_kqv.close()  # free KT/QT/V
        pool_xo = s_xo.enter_context(tc.tile_pool(name="xo", bufs=1))
        t_xo = pool_xo.tile([P, QT_, C], F32)
        pool_h2T = s_h2T.enter_context(tc.tile_pool(name="h2T", bufs=1))
        t_h2T = pool_h2T.tile([P, CT, N // 2], F32R)
        with ExitStack() as ph:
            lnp = ph.enter_context(tc.tile_pool(name="ln2", bufs=2))
            wst = ph.enter_context(tc.tile_pool(name="wst2", bufs=1))
            psD = ph.enter_context(tc.tile_pool(name="psD", bufs=4, space="PSUM"))
            psTr = ph.enter_context(tc.tile_pool(name="psTr2", bufs=3, space="PSUM"))
            t_wp = wst.tile([P, CT, C], F32R, tag="wp")
            nc.sync.dma_start(t_wp[:], wp[:].rearrange("c p n -> p c n"))
            for qt in range(QT_):
                xt = lnp.tile([P, C], F32, tag="xres")
                nc.sync.dma_start(xt[:], x_in[qt * P:(qt + 1) * P, :])
                for nc2 in range(2):
                    ns = slice(nc2 * 384, (nc2 + 1) * 384)
                    ps = psD.tile([P, 384], F32, tag="mm")
                    for fc in range(CT):
                        nc.tensor.matmul(
                            ps[:], t_OT[:, fc, qt * P:(qt + 1) * P], t_wp[:, fc, ns],
                            start=(fc == 0), stop=(fc == CT - 1))
                    nc.vector.tensor_tensor(
                        out=t_xo[:, qt, ns], in0=ps[:], in1=xt[:, ns], op=OP.add)
                if has_bpo:
                    nc.vector.tensor_tensor(
                        out=t_xo[:, qt, :], in0=t_xo[:, qt, :], in1=t_bpo[:], op=OP.add)
                h2 = lnp.tile([P, C], F32R, tag="h2")
                layernorm(lnp, t_xo[:, qt], h2, t_eps)
                for c in range(CT):
                    pst = psTr.tile([P, P], F32R, tag="tr2")
                    nc.tensor.transpose(pst[:], h2[:, c * P:(c + 1) * P], t_id[:])
                    with nc.allow_low_precision(reason="fp32r"):
                        nc.vector.tensor_copy(
                            out=t_h2T[:, c, qt * P:(qt + 1) * P], in_=pst[:])

        # ---------- Phase E: MLP ----------
        s_OT.close()  # free OT
        gtp = s_gT.enter_context(tc.tile_pool(name="gT", bufs=1, side="right"))
        t_gT = gtp.tile([P, FT, N // 2], F32R)
        with ExitStack() as ph:
            w1st = ph.enter_context(tc.tile_pool(name="w1st", bufs=2))
            psE = ph.enter_context(tc.tile_pool(name="psE", bufs=4, space="PSUM"))
            for f in range(FT):
                t_w1 = w1st.tile([P, CT, P], F32R, tag="w1")
                nc.sync.dma_start(t_w1[:], w1[f])
                for qc in range(2):
                    ps = psE.tile([P, 512], F32, tag="mm1")
                    for c in range(CT):
                        nc.tensor.matmul(
                            ps[:], t_w1[:, c], t_h2T[:, c, qc * 512:(qc + 1) * 512],
                            start=(c == 0), stop=(c == CT - 1))
                    nc.scalar.activation(
                        out=t_gT[:, f, qc * 512:(qc + 1) * 512], in_=ps[:],
                        func=AF.Gelu, bias=t_b1[:, f:f + 1])
        s_h2T.close()  # free h2T
        # fc2 in 3 chunks of 8 ff-tiles, accumulated into xo
        with ExitStack() as ph:
            w2st = ph.enter_context(tc.tile_pool(name="w2st", bufs=2))
            psF = ph.enter_context(tc.tile_pool(name="psF", bufs=4, space="PSUM"))
            NCH = 3
            FPC = FT // NCH
            for ch in range(NCH):
                t_w2 = w2st.tile([P, FPC, C], F32R, tag="w2")
                nc.sync.dma_start(
                    t_w2[:], w2[ch * FPC:(ch + 1) * FPC].rearrange("f p n -> p f n"))
                for qt in range(QT_):
                    for nc2 in range(2):
                        ns = slice(nc2 * 384, (nc2 + 1) * 384)
                        ps = psF.tile([P, 384], F32, tag="mm2")
                        for f in range(FPC):
                            nc.tensor.matmul(
                                ps[:], t_gT[:, ch * FPC + f, qt * P:(qt + 1) * P],
                                t_w2[:, f, ns],
                                start=(f == 0), stop=(f == FPC - 1))
                        nc.vector.tensor_tensor(
                            out=t_xo[:, qt, ns], in0=ps[:], in1=t_xo[:, qt, ns], op=OP.add)
            for qt in range(QT_):
                if has_bo:
                    nc.vector.tensor_tensor(
                        out=t_xo[:, qt, :], in0=t_xo[:, qt, :], in1=t_bo[:], op=OP.add)
                nc.sync.dma_start(y[qt * P:(qt + 1) * P, :], t_xo[:, qt])

    nc.compile()
    return nc


def kernel(**inputs):
    global LAST_RESULT
    from concourse.bass_utils import run_bass_kernel_spmd

    x = np.asarray(inputs["x"], dtype=np.float32)
    ln1_g = np.asarray(inputs["ln1_g"], np.float32)
    ln1_b = np.asarray(inputs["ln1_b"], np.float32)
    w_qkv = np.asarray(inputs["w_qkv"], np.float32)
    w_proj = np.asarray(inputs["w_proj"], np.float32)
    b_proj = np.asarray(inputs["b_proj"], np.float32)
    ln2_g = np.asarray(inputs["ln2_g"], np.float32)
    ln2_b = np.asarray(inputs["ln2_b"], np.float32)
    w1 = np.asarray(inputs["w1"], np.float32)
    b1 = np.asarray(inputs["b1"], np.float32)
    w2 = np.asarray(inputs["w2"], np.float32)
    b2 = np.asarray(inputs["b2"], np.float32)

    # Fold LN affine params into the weights (exact algebra; see module docstring)
    w_qkv_eff = w_qkv * ln1_g[:, None]
    qkv_bias = ln1_b @ w_qkv                     # [3C]
    q_bias = qkv_bias[:C]                        # added to Q features
    vb = qkv_bias[2 * C:]                        # V bias -> folds into proj bias
    bpo = b_proj + vb @ w_proj                   # [C]
    w1_eff = w1 * ln2_g[:, None]
    b1_eff = b1 + ln2_b @ w1                     # [FF], applied in gelu
    has_bpo = bool(np.any(bpo != 0))
    has_bo = bool(np.any(b2 != 0))

    key = (has_bpo, has_bo)
    if key not in _CACHE:
        _CACHE[key] = _build(has_bpo, has_bo)
    nc = _CACHE[key]

    wq_h = np.ascontiguousarray(
        w_qkv_eff[:, :C].reshape(CT, P, CT, P).transpose(2, 1, 0, 3))
    wk_h = np.ascontiguousarray(
        w_qkv_eff[:, C:2 * C].reshape(CT, P, CT, P).transpose(2, 1, 0, 3))
    wv_h = np.ascontiguousarray(w_qkv_eff[:, 2 * C:].reshape(CT, P, C))
    wp_h = np.ascontiguousarray(w_proj.reshape(CT, P, C))
    w1_h = np.ascontiguousarray(
        w1_eff.reshape(CT, P, FT, P).transpose(2, 1, 0, 3))
    w2_h = np.ascontiguousarray(w2.reshape(FT, P, C))
    qb_h = np.ascontiguousarray(q_bias.reshape(CT, P).T)
    b1_h = np.ascontiguousarray(b1_eff.reshape(FT, P).T)

    shared = {
        "wq": wq_h, "wk": wk_h, "wv": wv_h, "wp": wp_h, "w1": w1_h, "w2": w2_h,
        "qb": qb_h, "b1v": b1_h,
        "bpo": bpo.astype(np.float32), "bo": b2.astype(np.float32),
        "ident": np.eye(P, dtype=np.float32),
        "ones64": np.ones((1, HD), np.float32),
        "onesc": np.ones((P, NT * HEADS), np.float32),
    }
    in_maps = []
    for core in range(8):
        b, half = core // 2, core % 2
        own = x[b, half * 1024:(half + 1) * 1024]
        other = x[b, (1 - half) * 1024:(2 - half) * 1024]
        x_c = np.ascontiguousarray(np.concatenate([own, other], axis=0))
        in_maps.append(dict(shared, x_in=x_c))

    trace = os.environ.get("KERNEL_TRACE", "0") == "1"
    res = run_bass_kernel_spmd(nc, in_maps, core_ids=list(range(8)), trace=trace)
    LAST_RESULT = res

    out = np.empty((B, N, C), dtype=np.float32)
    for core in range(8):
        b, half = core // 2, core % 2
        out[b, half * 1024:(half + 1) * 1024] = res.results[core]["y"]
    return out


# revision 17
# speedup vs baseline: 1.2190x; 1.2190x over previous
"""Trainium2 Bass kernel for a pre-norm transformer encoder block.

Problem shapes (hardcoded): x [4, 2048, 768], 12 heads x 64, d_ff 3072.

Sharding: 8 cores, no collectives. Core c handles batch b = c // 2 and the
token half h = c % 2 (1024 "own" tokens). Each core receives the full 2048
tokens of its batch (own half first) so it can compute K/V locally; Q and
everything downstream (proj, MLP, output) run on its 1024 own tokens only.

Per-core on-chip schedule (all matmuls in float32r = full PE speed, ~1e-4 rel):
  A. LN1 over 2048 tokens, PE-transpose h -> hT [768, 2048]
  B. QKV: K^T,Q^T feature-major; V token-major staged as [keys, head, 64+1]
     with a ones column (ones-trick: PV matmul also yields softmax sums)
  C. attention per (head, q-chunk): S^T = kT.T @ qT (keys on partitions),
     P = exp(S/8) with no max-subtraction (scores are bounded ~8), O'^T//sums
     via ones-augmented PV + K=1 broadcast matmul for the normalization
  D. proj + residual -> xo, LN2, transpose h2 -> h2T
  E. MLP: g^T = gelu(w1.T @ h2T) feature-major, fc2 accumulated into xo
"""

import os
import sys
import types

import numpy as np

# This image's antenv lacks ``axon_hooks``, so the boot shim can't register
# the NTFF-profiling hook and trace=True silently degrades. Provide the
# registry module with a lazily-built ctypes hook against libaxon_pjrt.so
# (mirrors trn_agent_boot.trn_boot._ntff_profile_via_ctypes).
if "antenv.axon_hooks" not in sys.modules:
    _m = types.ModuleType("antenv.axon_hooks")
    _m._hook = None

    def _build_ctypes_hook():
        import contextlib
        import ctypes

        so_path = "/opt/axon/libaxon_pjrt.so"
        if not os.path.exists(so_path):
            return None
        lib = ctypes.CDLL(so_path)
        if not hasattr(lib, "axon_start_nrt_profile"):
            return None
        lib.axon_start_nrt_profile.argtypes = [
            ctypes.POINTER(ctypes.c_int64), ctypes.c_size_t]
        lib.axon_start_nrt_profile.restype = ctypes.c_int64
        lib.axon_stop_nrt_profile.argtypes = [ctypes.c_char_p]
        lib.axon_stop_nrt_profile.restype = ctypes.c_int64

        @contextlib.contextmanager
        def _hook(output_dir, device_ids):
            import jax
            jax.devices()
            if device_ids:
                ids = (ctypes.c_int64 * len(device_ids))(*device_ids)
                rc = lib.axon_start_nrt_profile(ids, len(device_ids))
            else:
                rc = lib.axon_start_nrt_profile(None, 0)
            if rc != 0:
                raise RuntimeError(f"axon_start_nrt_profile rc={rc}")
            try:
                yield
            finally:
                n = lib.axon_stop_nrt_profile(str(output_dir).encode())
                if n < 0:
                    raise RuntimeError(f"axon_stop_nrt_profile rc={n}")
                print(f"profile: {n} file(s) written to {output_dir}")

        return _hook

    def _set(h, _m=_m):
        _m._hook = h

    def _get(_m=_m):
        if _m._hook is None:
            _m._hook = _build_ctypes_hook()
        return _m._hook

    _m.set_axon_ntff_profile_hook = _set
    _m.get_axon_ntff_profile_hook = _get
    sys.modules["antenv.axon_hooks"] = _m

B, N, C = 4, 2048, 768
HEADS, HD = 12, 64
FF = 4 * C
P = 128
NT = N // P            # 16 token tiles (full context)
QT_ = (N // 2) // P    # 8 own token tiles
CT = C // P            # 6 feature tiles
FT = FF // P           # 24 ff tiles
LN_EPS = 1e-5

_CACHE = {}
LAST_RESULT = None


def _build(has_bpo, has_bo):
    import concourse.bass as bass
    import concourse.mybir as mybir
    import concourse.tile as tile
    from concourse import bacc
    from contextlib import ExitStack

    F32 = mybir.dt.float32
    F16 = mybir.dt.float16
    AF = mybir.ActivationFunctionType
    OP = mybir.AluOpType

    nc = bacc.Bacc(None, target_bir_lowering=False)

    # ---- DRAM tensors ----
    x_in = nc.dram_tensor("x_in", [N, C], F32, kind="ExternalInput")
    wq = nc.dram_tensor("wq", [CT, P, CT, P], F16, kind="ExternalInput")
    wk = nc.dram_tensor("wk", [CT, P, CT, P], F16, kind="ExternalInput")
    wv = nc.dram_tensor("wv", [CT, P, C], F16, kind="ExternalInput")
    wp = nc.dram_tensor("wp", [CT, P, C], F16, kind="ExternalInput")
    w1 = nc.dram_tensor("w1", [FT, P, CT, P], F16, kind="ExternalInput")
    w2 = nc.dram_tensor("w2", [FT, P, C], F16, kind="ExternalInput")
    qb = nc.dram_tensor("qb", [P, CT], F32, kind="ExternalInput")
    b1v = nc.dram_tensor("b1v", [P, FT], F32, kind="ExternalInput")
    bpo = nc.dram_tensor("bpo", [C], F32, kind="ExternalInput")
    bo = nc.dram_tensor("bo", [C], F32, kind="ExternalInput")
    ident = nc.dram_tensor("ident", [P, P], F16, kind="ExternalInput")
    ones64 = nc.dram_tensor("ones64", [1, HD], F16, kind="ExternalInput")
    onesc = nc.dram_tensor("onesc", [P, NT * HEADS], F16, kind="ExternalInput")
    y = nc.dram_tensor("y", [N // 2, C], F32, kind="ExternalOutput")

    def bcast_rows(t):
        # DRAM [C] -> AP broadcasting along 128 partitions
        return bass.AP(tensor=t.tensor, offset=t.offset, ap=[[0, P], list(t.ap[0])])

    def layernorm(pool, xt, ht, t_eps):
        # xt [128, 768] f32 -> ht [128, 768] f32r, normalized (no scale/bias)
        stats = pool.tile([P, 3, nc.vector.BN_STATS_DIM], F32, tag="ln_stats")
        for sg in range(3):
            nc.vector.bn_stats(out=stats[:, sg], in_=xt[:, sg * 256:(sg + 1) * 256])
        mv = pool.tile([P, nc.vector.BN_AGGR_DIM], F32, tag="ln_mv")
        nc.vector.bn_aggr(out=mv[:], in_=stats[:])
        std = pool.tile([P, 1], F32, tag="ln_std")
        nc.scalar.activation(out=std[:], in_=mv[:, 1:2], func=AF.Sqrt, bias=t_eps[:])
        rstd = pool.tile([P, 1], F32, tag="ln_rstd")
        nc.vector.reciprocal(out=rstd[:], in_=std[:])
        with nc.allow_low_precision(reason="fp32r rounding for matmul input"):
            nc.vector.tensor_scalar(
                out=ht[:], in0=xt[:], scalar1=mv[:, 0:1], scalar2=rstd[:],
                op0=OP.subtract, op1=OP.mult)

    with tile.TileContext(nc) as tc, ExitStack() as top:
        consts = top.enter_context(tc.tile_pool(name="consts", bufs=1))
        t_id = consts.tile([P, P], F16)
        nc.sync.dma_start(t_id[:], ident[:])
        t_ones64 = consts.tile([1, HD], F16)
        nc.sync.dma_start(t_ones64[:], ones64[:])
        t_qb = consts.tile([P, CT], F32)
        nc.sync.dma_start(t_qb[:], qb[:])
        t_b1 = consts.tile([P, FT], F32)
        nc.sync.dma_start(t_b1[:], b1v[:])
        t_eps = consts.tile([P, 1], F32)
        nc.vector.memset(t_eps[:], LN_EPS)
        t_bpo = t_bo = None
        if has_bpo:
            t_bpo = consts.tile([P, C], F32)
            nc.sync.dma_start(t_bpo[:], bcast_rows(bpo[:]))
        if has_bo:
            t_bo = consts.tile([P, C], F32)
            nc.sync.dma_start(t_bo[:], bcast_rows(bo[:]))

        s_kqv = ExitStack()   # closes after attention
        s_OT = ExitStack()    # closes after proj
        s_xo = ExitStack()    # closes at end
        s_h2T = ExitStack()   # closes after fc1
        s_gT = ExitStack()    # closes at end
        top.enter_context(s_gT)
        top.enter_context(s_xo)

        pool_kqv = s_kqv.enter_context(tc.tile_pool(name="kqv", bufs=1))
        t_KT = pool_kqv.tile([P, CT, N], F16)        # K^T feature-major
        t_QT = pool_kqv.tile([P, CT, N // 2], F16)   # Q^T own tokens
        t_V = pool_kqv.tile([P, NT, HEADS, HD + 1], F16)  # V + ones col

        # ---------- Phase A+B: LN1 + transpose + QKV ----------
        with ExitStack() as ph:
            lnp = ph.enter_context(tc.tile_pool(name="ln1", bufs=2))
            wst = ph.enter_context(tc.tile_pool(name="wst", bufs=2))
            hTp = ph.enter_context(tc.tile_pool(name="hT", bufs=1))
            psA = ph.enter_context(tc.tile_pool(name="psA", bufs=3, space="PSUM"))
            psTr = ph.enter_context(tc.tile_pool(name="psTr", bufs=2, space="PSUM"))

            # ones columns of the V staging buffer, one DMA
            nc.sync.dma_start(
                t_V[:, :, :, HD:HD + 1],
                onesc[:].rearrange("p (t h) -> p t h", t=NT)[:, :, :, None])

            t_wv = wst.tile([P, CT, C], F16, tag="wv")
            nc.sync.dma_start(t_wv[:], wv[:].rearrange("c p n -> p c n"))

            for g in range(2):  # token groups of 1024 (g=0: own tokens)
                t_hT = hTp.tile([P, CT, N // 2], F16, tag="hT")
                for tt in range(QT_):
                    t = g * QT_ + tt
                    xt = lnp.tile([P, C], F32, tag="xt")
                    nc.sync.dma_start(xt[:], x_in[t * P:(t + 1) * P, :])
                    ht = lnp.tile([P, C], F16, tag="ht")
                    layernorm(lnp, xt, ht, t_eps)
                    for c in range(CT):
                        pst = psTr.tile([P, P], F16, tag="tr")
                        nc.tensor.transpose(pst[:], ht[:, c * P:(c + 1) * P], t_id[:])
                        with nc.allow_low_precision(reason="fp32r"):
                            nc.vector.tensor_copy(
                                out=t_hT[:, c, tt * P:(tt + 1) * P], in_=pst[:])
                # K^T (this token group's columns)
                for f in range(CT):
                    t_wk = wst.tile([P, CT, P], F16, tag="wk")
                    nc.sync.dma_start(t_wk[:], wk[f])
                    for tc2 in range(2):
                        ps = psA.tile([P, 512], F32, tag="mm")
                        for c in range(CT):
                            nc.tensor.matmul(
                                ps[:], t_wk[:, c], t_hT[:, c, tc2 * 512:(tc2 + 1) * 512],
                                start=(c == 0), stop=(c == CT - 1))
                        with nc.allow_low_precision(reason="fp32r"):
                            nc.vector.tensor_copy(
                                out=t_KT[:, f, g * 1024 + tc2 * 512:g * 1024 + (tc2 + 1) * 512],
                                in_=ps[:])
                # Q^T (own tokens only)
                if g == 0:
                    for f in range(CT):
                        t_wq = wst.tile([P, CT, P], F16, tag="wq")
                        nc.sync.dma_start(t_wq[:], wq[f])
                        for tc2 in range(2):
                            ps = psA.tile([P, 512], F32, tag="mm")
                            for c in range(CT):
                                nc.tensor.matmul(
                                    ps[:], t_wq[:, c], t_hT[:, c, tc2 * 512:(tc2 + 1) * 512],
                                    start=(c == 0), stop=(c == CT - 1))
                            with nc.allow_low_precision(reason="fp32r"):
                                nc.vector.tensor_scalar(
                                    out=t_QT[:, f, tc2 * 512:(tc2 + 1) * 512], in0=ps[:],
                                    scalar1=t_qb[:, f:f + 1], scalar2=None, op0=OP.add)
                # V token-major, staged per head with ones column
                for tt in range(QT_):
                    t = g * QT_ + tt
                    for nc2 in range(2):
                        ps = psA.tile([P, 384], F32, tag="mmv")
                        for c in range(CT):
                            nc.tensor.matmul(
                                ps[:], t_hT[:, c, tt * P:(tt + 1) * P],
                                t_wv[:, c, nc2 * 384:(nc2 + 1) * 384],
                                start=(c == 0), stop=(c == CT - 1))
                        with nc.allow_low_precision(reason="fp32r"):
                            nc.vector.tensor_copy(
                                out=t_V[:, t, 6 * nc2:6 * nc2 + 6, :HD],
                                in_=ps[:].rearrange("p (h d) -> p h d", d=HD))

        # ---------- Phase C: attention ----------
        pool_OT = s_OT.enter_context(tc.tile_pool(name="OT", bufs=1, side="right"))
        t_OT = pool_OT.tile([P, CT, N // 2], F16)
        with ExitStack() as ph:
            ptp = ph.enter_context(tc.tile_pool(name="pt", bufs=4))
            rbp = ph.enter_context(tc.tile_pool(name="rb", bufs=2))
            psS = ph.enter_context(tc.tile_pool(name="psS", bufs=3, space="PSUM"))
            psO = ph.enter_context(tc.tile_pool(name="psO", bufs=2, space="PSUM"))
            psB = ph.enter_context(tc.tile_pool(name="psB", bufs=1, space="PSUM"))
            for hp in range(HEADS // 2):
                for qc in range(2):
                    qs = slice(qc * 512, (qc + 1) * 512)
                    pso = {}
                    for sub in range(2):
                        pso[sub] = psO.tile(
                            [HD + 1, 512], F32, tag=f"o{sub}", name=f"pso{sub}")
                    for kt in range(NT):
                        for sub in range(2):
                            h = 2 * hp + sub
                            off = sub * HD
                            ps = psS.tile([P, 512], F32, tag="s")
                            nc.tensor.matmul(
                                ps[:], t_KT[off:off + HD, hp, kt * P:(kt + 1) * P],
                                t_QT[off:off + HD, hp, qs], start=True, stop=True)
                            pt = ptp.tile([P, 512], F16, tag=f"pt{sub}")
                            nc.scalar.activation(
                                out=pt[:], in_=ps[:], func=AF.Exp, scale=0.125)
                            nc.tensor.matmul(
                                pso[sub][:], t_V[:, kt, h, :], pt[:],
                                start=(kt == 0), stop=(kt == NT - 1))
                    for sub in range(2):
                        off = sub * HD
                        sums = rbp.tile([1, 512], F32, tag="sums")
                        nc.vector.tensor_copy(out=sums[:], in_=pso[sub][HD:HD + 1, :])
                        r32 = rbp.tile([1, 512], F32, tag="r32")
                        nc.vector.reciprocal_approx_fast(out=r32[:], in_=sums[:])
                        # scale by 4096 to keep 1/sums in fp16 normal range;
                        # the broadcast ones vector is 1/4096 to compensate
                        r = rbp.tile([1, 512], F16, tag="r")
                        with nc.allow_low_precision(reason="fp16 matmul input"):
                            nc.vector.tensor_scalar_mul(
                                out=r[:], in0=r32[:], scalar1=4096.0)
                        pb = psB.tile([HD, 512], F32, tag="b")
                        nc.tensor.matmul(pb[:], t_ones64[:], r[:], start=True, stop=True)
                        rb = rbp.tile([HD, 512], F32, tag=f"rb{sub}")
                        nc.any.tensor_copy(out=rb[:], in_=pb[:])
                        with nc.allow_low_precision(reason="fp16 matmul input"):
                            nc.vector.tensor_tensor(
                                out=t_OT[off:off + HD, hp, qs], in0=pso[sub][:HD, :],
                                in1=rb[:], op=OP.mult)

        # ---------- Phase D: proj + residual + LN2 + transpose ----------
        s_kqv.close()  # free KT/QT/V
        pool_xo = s_xo.enter_context(tc.tile_pool(name="xo", bufs=1))
        t_xo = pool_xo.tile([P, QT_, C], F32)
        pool_h2T = s_h2T.enter_context(tc.tile_pool(name="h2T", bufs=1))
        t_h2T = pool_h2T.tile([P, CT, N // 2], F16)
        with ExitStack() as ph:
            lnp = ph.enter_context(tc.tile_pool(name="ln2", bufs=2))
            wst = ph.enter_context(tc.tile_pool(name="wst2", bufs=1))
            psD = ph.enter_context(tc.tile_pool(name="psD", bufs=4, space="PSUM"))
            psTr = ph.enter_context(tc.tile_pool(name="psTr2", bufs=3, space="PSUM"))
            t_wp = wst.tile([P, CT, C], F16, tag="wp")
            nc.sync.dma_start(t_wp[:], wp[:].rearrange("c p n -> p c n"))
            for qt in range(QT_):
                xt = lnp.tile([P, C], F32, tag="xres")
                nc.sync.dma_start(xt[:], x_in[qt * P:(qt + 1) * P, :])
                for nc2 in range(2):
                    ns = slice(nc2 * 384, (nc2 + 1) * 384)
                    ps = psD.tile([P, 384], F32, tag="mm")
                    for fc in range(CT):
                        nc.tensor.matmul(
                            ps[:], t_OT[:, fc, qt * P:(qt + 1) * P], t_wp[:, fc, ns],
                            start=(fc == 0), stop=(fc == CT - 1))
                    nc.vector.tensor_tensor(
                        out=t_xo[:, qt, ns], in0=ps[:], in1=xt[:, ns], op=OP.add)
                if has_bpo:
                    nc.vector.tensor_tensor(
                        out=t_xo[:, qt, :], in0=t_xo[:, qt, :], in1=t_bpo[:], op=OP.add)
                h2 = lnp.tile([P, C], F16, tag="h2")
                layernorm(lnp, t_xo[:, qt], h2, t_eps)
                for c in range(CT):
                    pst = psTr.tile([P, P], F16, tag="tr2")
                    nc.tensor.transpose(pst[:], h2[:, c * P:(c + 1) * P], t_id[:])
                    with nc.allow_low_precision(reason="fp32r"):
                        nc.vector.tensor_copy(
                            out=t_h2T[:, c, qt * P:(qt + 1) * P], in_=pst[:])

        # ---------- Phase E: MLP ----------
        s_OT.close()  # free OT
        gtp = s_gT.enter_context(tc.tile_pool(name="gT", bufs=1, side="right"))
        t_gT = gtp.tile([P, FT, N // 2], F16)
        with ExitStack() as ph:
            w1st = ph.enter_context(tc.tile_pool(name="w1st", bufs=2))
            psE = ph.enter_context(tc.tile_pool(name="psE", bufs=2, space="PSUM"))
            for f in range(FT):
                t_w1 = w1st.tile([P, CT, P], F16, tag="w1")
                nc.sync.dma_start(t_w1[:], w1[f])
                ps = psE.tile([P, 1024], F32, tag="mm1")
                for qc in range(2):
                    for c in range(CT):
                        nc.tensor.matmul(
                            ps[:, qc * 512:(qc + 1) * 512], t_w1[:, c],
                            t_h2T[:, c, qc * 512:(qc + 1) * 512],
                            start=(c == 0), stop=(c == CT - 1))
                nc.scalar.activation(
                    out=t_gT[:, f, :], in_=ps[:],
                    func=AF.Gelu, bias=t_b1[:, f:f + 1])
        s_h2T.close()  # free h2T
        # fc2 in 3 chunks of 8 ff-tiles, accumulated into xo
        with ExitStack() as ph:
            w2st = ph.enter_context(tc.tile_pool(name="w2st", bufs=2))
            psF = ph.enter_context(tc.tile_pool(name="psF", bufs=4, space="PSUM"))
            NCH = 3
            FPC = FT // NCH
            for ch in range(NCH):
                t_w2 = w2st.tile([P, FPC, C], F16, tag="w2")
                nc.sync.dma_start(
                    t_w2[:], w2[ch * FPC:(ch + 1) * FPC].rearrange("f p n -> p f n"))
                for qt in range(QT_):
                    for nc2 in range(2):
                        ns = slice(nc2 * 384, (nc2 + 1) * 384)
                        ps = psF.tile([P, 384], F32, tag="mm2")
                        for f in range(FPC):
                            nc.tensor.matmul(
                                ps[:], t_gT[:, ch * FPC + f, qt * P:(qt + 1) * P],
                                t_w2[:, f, ns],
                                start=(f == 0), stop=(f == FPC - 1))
                        nc.vector.tensor_tensor(
                            out=t_xo[:, qt, ns], in0=ps[:], in1=t_xo[:, qt, ns], op=OP.add)
            for qt in range(QT_):
                if has_bo:
                    nc.vector.tensor_tensor(
                        out=t_xo[:, qt, :], in0=t_xo[:, qt, :], in1=t_bo[:], op=OP.add)
                nc.sync.dma_start(y[qt * P:(qt + 1) * P, :], t_xo[:, qt])

    nc.compile()
    return nc


def kernel(**inputs):
    global LAST_RESULT
    from concourse.bass_utils import run_bass_kernel_spmd

    x = np.asarray(inputs["x"], dtype=np.float32)
    ln1_g = np.asarray(inputs["ln1_g"], np.float32)
    ln1_b = np.asarray(inputs["ln1_b"], np.float32)
    w_qkv = np.asarray(inputs["w_qkv"], np.float32)
    w_proj = np.asarray(inputs["w_proj"], np.float32)
    b_proj = np.asarray(inputs["b_proj"], np.float32)
    ln2_g = np.asarray(inputs["ln2_g"], np.float32)
    ln2_b = np.asarray(inputs["ln2_b"], np.float32)
    w1 = np.asarray(inputs["w1"], np.float32)
    b1 = np.asarray(inputs["b1"], np.float32)
    w2 = np.asarray(inputs["w2"], np.float32)
    b2 = np.asarray(inputs["b2"], np.float32)

    # Fold LN affine params into the weights (exact algebra; see module docstring)
    w_qkv_eff = w_qkv * ln1_g[:, None]
    qkv_bias = ln1_b @ w_qkv                     # [3C]
    q_bias = qkv_bias[:C]                        # added to Q features
    vb = qkv_bias[2 * C:]                        # V bias -> folds into proj bias
    bpo = b_proj + vb @ w_proj                   # [C]
    w1_eff = w1 * ln2_g[:, None]
    b1_eff = b1 + ln2_b @ w1                     # [FF], applied in gelu
    has_bpo = bool(np.any(bpo != 0))
    has_bo = bool(np.any(b2 != 0))

    key = (has_bpo, has_bo)
    if key not in _CACHE:
        _CACHE[key] = _build(has_bpo, has_bo)
    nc = _CACHE[key]

    f16 = np.float16
    wq_h = np.ascontiguousarray(
        w_qkv_eff[:, :C].reshape(CT, P, CT, P).transpose(2, 1, 0, 3)).astype(f16)
    wk_h = np.ascontiguousarray(
        w_qkv_eff[:, C:2 * C].reshape(CT, P, CT, P).transpose(2, 1, 0, 3)).astype(f16)
    wv_h = np.ascontiguousarray(w_qkv_eff[:, 2 * C:].reshape(CT, P, C)).astype(f16)
    wp_h = np.ascontiguousarray(w_proj.reshape(CT, P, C)).astype(f16)
    w1_h = np.ascontiguousarray(
        w1_eff.reshape(CT, P, FT, P).transpose(2, 1, 0, 3)).astype(f16)
    w2_h = np.ascontiguousarray(w2.reshape(FT, P, C)).astype(f16)
    qb_h = np.ascontiguousarray(q_bias.reshape(CT, P).T)
    b1_h = np.ascontiguousarray(b1_eff.reshape(FT, P).T)

    shared = {
        "wq": wq_h, "wk": wk_h, "wv": wv_h, "wp": wp_h, "w1": w1_h, "w2": w2_h,
        "qb": qb_h, "b1v": b1_h,
        "bpo": bpo.astype(np.float32), "bo": b2.astype(np.float32),
        "ident": np.eye(P, dtype=np.float16),
        "ones64": np.full((1, HD), 1.0 / 4096.0, np.float16),
        "onesc": np.ones((P, NT * HEADS), np.float16),
    }
    in_maps = []
    for core in range(8):
        b, half = core // 2, core % 2
        own = x[b, half * 1024:(half + 1) * 1024]
        other = x[b, (1 - half) * 1024:(2 - half) * 1024]
        x_c = np.ascontiguousarray(np.concatenate([own, other], axis=0))
        in_maps.append(dict(shared, x_in=x_c))

    trace = os.environ.get("KERNEL_TRACE", "0") == "1"
    res = run_bass_kernel_spmd(nc, in_maps, core_ids=list(range(8)), trace=trace)
    LAST_RESULT = res

    out = np.empty((B, N, C), dtype=np.float32)
    for core in range(8):
        b, half = core // 2, core % 2
        out[b, half * 1024:(half + 1) * 1024] = res.results[core]["y"]
    return out
